# revision 30
# baseline (speedup 1.0000x reference)
"""Trainium2 kernel for nn_LocalEncoder (BLT-style local encoder).

Key structural insight: every per-token quantity depends only on the token ID
(vocab=260), so the whole cross-attention collapses into vocab space:

  out_h(patch j) = sum_w C[w,j] * exp(S_h[w, qtok_j]) * vhat_h(w) / den
  den            = sum_w C[w,j] * exp(S_h[w, qtok_j])

with C[w,j] = count of tokens with id w inside patch j (host histogram),
S_h = khat_h^T qhat_h a (vocab x patch) score matrix, and qhat/khat/vhat the
vocab-space projection tables.  Device work per core is a handful of dense
vocab-sized matmuls -- no per-token gathers at all.

Pipeline:
  Kernel A (8 cores, DF split 8x384): zv partials = w2_slice @ silu(w1_slice @ embT)
  Host:     zv -> per-row boundary selection (stable by (-z, idx)) -> pos/pid,
            count matrix C[vocab, patch], qtok one-hot, folded weights
  Kernel B (8 cores = 4 seqs x 2 head-groups of 6): tables -> scores ->
            exp*count -> weighted-sum matmuls -> wo, partial outputs summed
            on host over the 2 head-groups.
"""

import os
import numpy as np
import ml_dtypes

import concourse.bass as bass
import concourse.bacc as bacc
import concourse.mybir as mybir
from concourse.tile import TileContext
from concourse.alu_op_type import AluOpType
from concourse.bass_utils import run_bass_kernel_spmd

F32 = mybir.dt.float32
F32R = mybir.dt.float32r
BF16 = mybir.dt.bfloat16
AFT = mybir.ActivationFunctionType
AX = mybir.AxisListType

B, L, D, V, K, H, HD = 4, 4096, 768, 260, 512, 12, 64
DF = 4 * D
VP = 384          # vocab padded to 3 partition chunks
RMS_EPS = 1e-5
NCORES = 8
FSL = DF // NCORES  # 384 f-rows per core in kernel A
DG = 384            # head-group width (6 heads x 64)

_cache = {}


# --------------------------------------------------------------------------- #
# Kernel A: per-core partial zv over a DF slice (fp32 matmuls for precision)
# --------------------------------------------------------------------------- #
def build_kernel_a():
    nc = bacc.Bacc("TRN2", target_bir_lowering=False, debug=False)
    embT_d = nc.dram_tensor("embT", [128, 6 * V], F32R, kind="ExternalInput")
    w1T_d = nc.dram_tensor("w1T", [128, 6 * FSL], F32R, kind="ExternalInput")
    bw_d = nc.dram_tensor("bw", [128, 6], F32, kind="ExternalInput")
    emb_d = nc.dram_tensor("emb", [128, 3 * D], BF16, kind="ExternalInput")
    wp_d = nc.dram_tensor("wp", [128, 6 * 192], BF16, kind="ExternalInput")
    wkp_d = nc.dram_tensor("wkp", [128, 6 * 96], BF16, kind="ExternalInput")
    zp_d = nc.dram_tensor("zp", [1, V], F32, kind="ExternalOutput")
    qv_d = nc.dram_tensor("qv", [128, 3 * 192], BF16, kind="ExternalOutput")
    kp_d = nc.dram_tensor("kp", [96, VP], BF16, kind="ExternalOutput")
    rv_d = nc.dram_tensor("rv", [128, 3], F32, kind="ExternalOutput")

    with TileContext(nc) as tc:
        with (
            tc.tile_pool(name="sb", bufs=1) as sb,
            tc.tile_pool(name="ps", bufs=2, space="PSUM") as ps,
        ):
            embT_t = sb.tile([128, 6 * V], F32R, tag="embT", name="embT_t")
            w1T_t = sb.tile([128, 6 * FSL], F32R, tag="w1T", name="w1T_t")
            bw = sb.tile([128, 6], F32, tag="bw", name="bw")
            emb_t = sb.tile([128, 3 * D], BF16, tag="emb", name="emb_t")
            wp_t = sb.tile([128, 6 * 192], BF16, tag="wp", name="wp_t")
            wkp_t = sb.tile([128, 6 * 96], BF16, tag="wkp", name="wkp_t")
            nc.sync.dma_start(bw[:, :], bw_d[:, :])
            for u in range(3):
                nc.sync.dma_start(emb_t[:, D * u:D * (u + 1)],
                                  emb_d[:, D * u:D * (u + 1)])
            for d in range(6):
                nc.sync.dma_start(embT_t[:, V * d:V * (d + 1)],
                                  embT_d[:, V * d:V * (d + 1)])
                nc.sync.dma_start(w1T_t[:, FSL * d:FSL * (d + 1)],
                                  w1T_d[:, FSL * d:FSL * (d + 1)])
            nc.sync.dma_start(wp_t[:, :], wp_d[:, :])
            nc.sync.dma_start(wkp_t[:, :], wkp_d[:, :])
            embT = [embT_t[:, V * d:V * (d + 1)] for d in range(6)]
            w1T = [w1T_t[:, FSL * d:FSL * (d + 1)] for d in range(6)]
            b1c = bw[:, 0:3]
            w2c = bw[:, 3:6]

            zp_ps = ps.tile([1, V], F32, tag="zp")
            for fi in range(3):
                y1p = ps.tile([128, V], F32, tag="y1")
                for d in range(6):
                    nc.tensor.matmul(
                        y1p[:, :], w1T_t[:, FSL * d + 128 * fi:FSL * d + 128 * (fi + 1)],
                        embT[d], start=(d == 0), stop=(d == 5),
                    )
                y1s = sb.tile([128, V], F32, tag="y1s")
                nc.scalar.activation(y1s[:, :], y1p[:, :], AFT.Silu,
                                     bias=b1c[:, fi:fi + 1])
                nc.tensor.matmul(zp_ps[:, :], w2c[:, fi:fi + 1], y1s[:, :],
                                 start=(fi == 0), stop=(fi == 2))
            zp_s = sb.tile([1, V], F32, tag="zps")
            nc.vector.tensor_copy(zp_s[:, :], zp_ps[:, :])
            nc.sync.dma_start(zp_d[:, :], zp_s[:, :])

            # bf16 copy of embT padded to VP columns per chunk
            embT_b = sb.tile([128, 6 * VP], BF16, tag="embTb", name="embT_b")
            nc.gpsimd.memset(embT_b[:, :], 0.0)
            for d in range(6):
                nc.vector.tensor_copy(embT_b[:, VP * d:VP * d + V], embT[d])

            # rv chain (bf16 emb squares -> rsqrt of mean)
            rv_t = sb.tile([128, 3], F32, tag="rv", name="rv_t")
            for u in range(3):
                sq = sb.tile([128, D], BF16, tag="sq", name="sq", bufs=2)
                nc.gpsimd.tensor_tensor(sq[:, :], emb_t[:, D * u:D * (u + 1)],
                                        emb_t[:, D * u:D * (u + 1)], AluOpType.mult)
                msq = sb.tile([128, 1], F32, tag="msq", name="msq", bufs=2)
                nc.vector.tensor_reduce(msq[:, :], sq[:, :], AX.X, AluOpType.add)
                tn = sb.tile([128, 1], F32, tag="tn", name="tn", bufs=2)
                nc.vector.tensor_scalar(tn[:, :], msq[:, :], 1.0 / D, RMS_EPS,
                                        AluOpType.mult, AluOpType.add)
                tr = sb.tile([128, 1], F32, tag="tr", name="tr", bufs=2)
                nc.vector.reciprocal(tr[:, :], tn[:, :])
                nc.scalar.activation(rv_t[:, u:u + 1], tr[:, :], AFT.Sqrt)
            nc.sync.dma_start(rv_d[:, :], rv_t[:, :])

            # table piece: qv = (emb @ w_cols) * rv ; kp = w_rowsT @ embT
            qv_s = sb.tile([128, 3 * 192], BF16, tag="qvs", name="qv_s")
            for u in range(3):
                qvp = ps.tile([128, 192], F32, tag="t192", name="qvp")
                for d in range(6):
                    nc.tensor.matmul(qvp[:, :],
                                     embT_b[:, VP * d + 128 * u:VP * d + 128 * (u + 1)],
                                     wp_t[:, 192 * d:192 * (d + 1)],
                                     start=(d == 0), stop=(d == 5))
                nc.vector.tensor_scalar(qv_s[:, 192 * u:192 * (u + 1)], qvp[:, :],
                                        rv_t[:, u:u + 1], None, AluOpType.mult)
            nc.sync.dma_start(qv_d[:, :], qv_s[:, :])
            kpp = ps.tile([96, VP], F32, tag="kpp", name="kpp")
            for d in range(6):
                nc.tensor.matmul(kpp[:, :], wkp_t[:, 96 * d:96 * (d + 1)],
                                 embT_b[:, VP * d:VP * (d + 1)],
                                 start=(d == 0), stop=(d == 5))
            kp_s = sb.tile([96, VP], BF16, tag="kps", name="kp_s")
            nc.vector.tensor_copy(kp_s[:, :], kpp[:, :])
            nc.sync.dma_start(kp_d[:, :], kp_s[:, :])

    nc.compile()
    return nc


def run_kernel_a(inputs, emb_s, wqT_full, wkT_full, wvT_full, pack):
    if "A" not in _cache:
        _cache["A"] = build_kernel_a()
    nc = _cache["A"]
    bf16 = ml_dtypes.bfloat16
    embT = inputs["embed_W"].astype(np.float32).T          # [768, 260]
    embT_r = np.ascontiguousarray(
        embT.reshape(6, 128, V).transpose(1, 0, 2).reshape(128, 6 * V))
    w1 = inputs["bp_w1"].astype(np.float32)
    b1 = inputs["bp_b1"].astype(np.float32)
    w2 = inputs["bp_w2"].astype(np.float32)[0]
    in_maps = []
    for c in range(NCORES):
        sl = slice(c * FSL, (c + 1) * FSL)
        w1T_r = np.ascontiguousarray(
            w1[sl].T.reshape(6, 128, FSL).transpose(1, 0, 2).reshape(128, 6 * FSL))
        bw = np.zeros((128, 6), np.float32)
        bw[:, 0:3] = b1[sl].reshape(3, 128).T
        bw[:, 3:6] = w2[sl].reshape(3, 128).T
        if c < 4:
            wp = wqT_full[:, 192 * c:192 * (c + 1)]
        else:
            wp = wvT_full[:, 192 * (c - 4):192 * (c - 3)]
        in_maps.append({
            "embT": embT_r, "w1T": w1T_r, "bw": bw, "emb": emb_s,
            "wp": pack(wp, 6).astype(bf16),
            "wkp": pack(wkT_full[:, 96 * c:96 * (c + 1)], 6).astype(bf16),
        })
    res = run_bass_kernel_spmd(nc, in_maps, list(range(NCORES)),
                               trace=os.environ.get("KERNEL_TRACE") == "1")
    _cache["tA"] = res.exec_time_ns
    zv = np.zeros(V, np.float64)
    for c in range(NCORES):
        zv += res.results[c]["zp"][0].astype(np.float64)
    zv += inputs["bp_b2"].astype(np.float64)[0]

    def unpack(a, nchunk):
        p, nc_ = a.shape
        c = nc_ // nchunk
        return a.reshape(p, nchunk, c).transpose(1, 0, 2).reshape(nchunk * p, c)

    qhat = np.zeros((VP, D), np.float32)
    vhat = np.zeros((VP, D), np.float32)
    ktT = np.zeros((D, VP), np.float32)
    for c in range(NCORES):
        r = res.results[c]
        qv = unpack(r["qv"].astype(np.float32), 3)
        if c < 4:
            qhat[:, 192 * c:192 * (c + 1)] = qv
        else:
            vhat[:, 192 * (c - 4):192 * (c - 3)] = qv
        ktT[96 * c:96 * (c + 1), :] = r["kp"].astype(np.float32)
    rv = res.results[0]["rv"]
    return zv.astype(np.float32), qhat, vhat, ktT, rv


# --------------------------------------------------------------------------- #
# Host boundary logic
# --------------------------------------------------------------------------- #
def boundary_plan(zv, tokens):
    """Reproduce reference top-k (stable ties by index) + patch structure."""
    zt = zv[tokens]  # [B, L]
    pos = np.zeros((B, K), np.int64)
    for b in range(B):
        key = zt[b].astype(np.float64).copy()
        key[0] = np.inf  # position 0 forced boundary (logprob set to 0 = max)
        order = np.lexsort((np.arange(L), -key))
        pos[b] = np.sort(order[:K])
    pid = (pos[:, None, :] <= np.arange(L)[None, :, None]).sum(-1) - 1  # [B, L]
    return pos, pid


# --------------------------------------------------------------------------- #
# Kernel B: count-matrix vocab-space cross attention, 6 heads per core
# --------------------------------------------------------------------------- #
def strided3(ap, n, w, stride, offset=0):
    """[128, *] AP viewed as [128, n, w] blocks at `offset` with block stride."""
    ps = ap.ap[0]
    return bass.AP(ap.tensor, ap.offset + offset,
                   [list(ps), [stride, n], [1, w]])


def build_kernel_b():
    nc = bacc.Bacc("TRN2", target_bir_lowering=False, debug=False)
    qgt_d = nc.dram_tensor("qgt", [128, 3 * K], BF16, kind="ExternalInput")
    ktT_d = nc.dram_tensor("ktT", [128, 3 * VP], BF16, kind="ExternalInput")
    vh_d = nc.dram_tensor("vh", [128, 3 * 390], BF16, kind="ExternalInput")
    rv_d = nc.dram_tensor("rv", [128, 3], F32, kind="ExternalInput")
    woT_d = nc.dram_tensor("woT", [128, 3 * D], BF16, kind="ExternalInput")
    c_d = nc.dram_tensor("cnt", [128, 3 * K], BF16, kind="ExternalInput")
    outT_d = nc.dram_tensor("outT", [D, K], BF16, kind="ExternalOutput")

    with TileContext(nc) as tc:
        with (
            tc.tile_pool(name="sb", bufs=1) as sb,
            tc.tile_pool(name="ps", bufs=2, space="PSUM") as ps,
        ):
            # ---- loads ----
            qgt_t = sb.tile([128, 3 * K], BF16, tag="qgt", name="qgt_t")
            ktT_t = sb.tile([128, 3 * VP], BF16, tag="ktT", name="ktT_t")
            vh_t = sb.tile([128, 3 * 390], BF16, tag="vh", name="vh_t")
            rv_t = sb.tile([128, 3], F32, tag="rvt", name="rv_t")
            c_t = sb.tile([128, 3 * K], BF16, tag="ct", name="c_t")
            wo_t = sb.tile([128, 3 * D], BF16, tag="wot", name="wo_t")
            nc.sync.dma_start(rv_t[:, :], rv_d[:, :])
            for u in range(3):
                nc.sync.dma_start(qgt_t[:, K * u:K * (u + 1)],
                                  qgt_d[:, K * u:K * (u + 1)])
                nc.sync.dma_start(ktT_t[:, VP * u:VP * (u + 1)],
                                  ktT_d[:, VP * u:VP * (u + 1)])
                nc.sync.dma_start(vh_t[:, 390 * u:390 * (u + 1)],
                                  vh_d[:, 390 * u:390 * (u + 1)])
                nc.sync.dma_start(c_t[:, K * u:K * (u + 1)],
                                  c_d[:, K * u:K * (u + 1)])
            for u in range(3):
                nc.sync.dma_start(wo_t[:, D * u:D * (u + 1)],
                                  woT_d[:, D * u:D * (u + 1)])
            ktT3 = [ktT_t[:, VP * r:VP * (r + 1)] for r in range(3)]
            vhat3 = [vh_t[:, 390 * u:390 * (u + 1)] for u in range(3)]
            C3 = [c_t[:, K * u:K * (u + 1)] for u in range(3)]
            woT3 = [wo_t[:, D * u:D * (u + 1)] for u in range(3)]
            rv3 = [rv_t[:, u:u + 1] for u in range(3)]
            ones64 = sb.tile([128, 64], BF16, tag="ones64")
            nc.gpsimd.memset(ones64[:, :], 1.0)

            # qgT DMA'd directly (host gathers qhat rows by qtok)
            qgT3 = [qgt_t[:, K * r:K * (r + 1)] for r in range(3)]

            # ---- attention: per head scores -> exp*count -> num/den ----
            prT3 = [sb.tile([128, K], BF16, tag=f"prT{r}", name=f"prT{r}") for r in range(3)]
            opart = [sb.tile([128, K], F32, tag=f"opart{m}", name=f"opart{m}")
                     for m in range(6)]
            for h in range(6):
                r, off = h // 2, 64 * (h % 2)
                nm = ps.tile([128, K], F32, tag="num", name="nm", bufs=2)
                for w in range(3):
                    sp = ps.tile([128, K], F32, tag="s3", name="sp", bufs=3)
                    nc.tensor.matmul(sp[:, :],
                                     ktT3[r][off:off + 64, 128 * w:128 * (w + 1)],
                                     qgT3[r][off:off + 64, :],
                                     start=True, stop=True)
                    ex = sb.tile([128, K], BF16, tag="ex", name="ex", bufs=4)
                    nc.scalar.activation(ex[:, :], sp[:, :], AFT.Exp,
                                         scale=rv3[w])
                    xt = sb.tile([128, K], BF16, tag="xt", name="xt", bufs=4)
                    nc.vector.tensor_tensor(xt[:, :], ex[:, :], C3[w][:, :],
                                            AluOpType.mult)
                    nc.tensor.matmul(nm[0:65, :], vhat3[w][:, 65 * h:65 * h + 65],
                                     xt[:, :], start=(w == 0), stop=(w == 2))
                rdf = sb.tile([128, K], F32, tag="rdf", name="rdf", bufs=3)
                nc.vector.reciprocal_approx_fast(rdf[:, :], nm[:, :])
                rd = sb.tile([128, K], BF16, tag="rd", name="rd", bufs=3)
                nc.vector.tensor_copy(rd[64:65, :], rdf[64:65, :])
                pp = ps.tile([128, K], F32, tag="pp", name="pp", bufs=1)
                nc.tensor.matmul(pp[0:64, :], ones64[64:65, :], rd[64:65, :],
                                 start=True, stop=True)
                nmsb = sb.tile([64, K], BF16, tag="nmsb", name="nmsb", bufs=3)
                nc.vector.tensor_copy(nmsb[:, :], nm[0:64, :])
                if off == 0:
                    nc.vector.tensor_tensor(prT3[r][0:64, :], nmsb[:, :],
                                            pp[0:64, :], AluOpType.mult)
                else:
                    po = sb.tile([64, K], BF16, tag="po", name="po", bufs=2)
                    nc.vector.tensor_tensor(po[:, :], nmsb[:, :], pp[0:64, :],
                                            AluOpType.mult)
                    nc.sync.dma_start(prT3[r][64:128, :], po[:, :])
                # wo projection wave: as soon as prT3[kc] completes (h odd),
                # fold its contribution into SBUF partials so only the last
                # wave sits in the tail
                if off == 64:
                    kc = r
                    for m in range(6):
                        op = ps.tile([128, K], F32, tag="w512", name="op")
                        nc.tensor.matmul(op[:, :],
                                         woT3[kc][:, 128 * m:128 * (m + 1)],
                                         prT3[kc][:, :], start=True, stop=True)
                        if kc == 0:
                            nc.vector.tensor_copy(opart[m][:, :], op[:, :])
                        elif kc == 1:
                            nc.vector.tensor_tensor(opart[m][:, :], opart[m][:, :],
                                                    op[:, :], AluOpType.add)
                        else:
                            ot = sb.tile([128, K], BF16, tag="ot", name="ot", bufs=2)
                            nc.vector.tensor_tensor(ot[:, :], opart[m][:, :],
                                                    op[:, :], AluOpType.add)
                            nc.sync.dma_start(outT_d[128 * m:128 * (m + 1), :],
                                              ot[:, :])
    nc.compile()
    return nc


# --------------------------------------------------------------------------- #
# top-level
# --------------------------------------------------------------------------- #
def kernel(tokens, embed_W, bp_w1, bp_b1, bp_w2, bp_b2, wq, wk, wv, wo,
           qnorm_w, kvnorm_w, k_patches):
    tokens = np.asarray(tokens).astype(np.int64)
    inputs = dict(tokens=tokens, embed_W=embed_W, bp_w1=bp_w1, bp_b1=bp_b1,
                  bp_w2=bp_w2, bp_b2=bp_b2)
    bf16 = ml_dtypes.bfloat16

    def pack(a, nchunk):
        """[nchunk*128, C] -> [128, nchunk*C] chunk-column layout."""
        n, c = a.shape
        assert n == nchunk * 128
        return np.ascontiguousarray(
            a.reshape(nchunk, 128, c).transpose(1, 0, 2).reshape(128, nchunk * c))

    embp = np.zeros((VP, D), np.float32)
    embp[:V] = embed_W.astype(np.float32)
    emb_s = pack(embp, 3).astype(bf16)
    wqT_full = np.ascontiguousarray(
        (wq.astype(np.float32) * qnorm_w.astype(np.float32)[None, :]).T / 8.0)
    wkT_full = np.ascontiguousarray(
        (wk.astype(np.float32) * kvnorm_w.astype(np.float32)[None, :]).T)
    wvT_full = np.ascontiguousarray(
        (wv.astype(np.float32) * kvnorm_w.astype(np.float32)[None, :]).T)
    woT_full = np.ascontiguousarray(wo.astype(np.float32).T)

    zv, qhat, vhat, ktT, rv = run_kernel_a(
        inputs, emb_s, wqT_full, wkT_full, wvT_full, pack)
    pos, pid = boundary_plan(zv, tokens)
    qtokp = np.take_along_axis(tokens, pos, 1)  # [B, K] boundary token ids

    if "B" not in _cache:
        _cache["B"] = build_kernel_b()
    nc = _cache["B"]

    qhat_b = qhat.astype(bf16).astype(np.float32)
    in_maps = []
    for b in range(B):
        C = np.zeros((VP, K), np.float32)
        np.add.at(C, (tokens[b], pid[b]), 1.0)
        C_s = pack(C, 3).astype(bf16)
        qg_b = qhat_b[qtokp[b]]                    # [K, 768] gather
        for g in range(2):
            cols = slice(DG * g, DG * (g + 1))
            vh390 = np.zeros((VP, 390), np.float32)
            for h in range(6):
                vh390[:, 65 * h:65 * h + 64] = vhat[:, DG * g + 64 * h:DG * g + 64 * (h + 1)]
                vh390[:, 65 * h + 64] = 1.0
            in_maps.append({
                "qgt": pack(np.ascontiguousarray(qg_b[:, cols].T), 3).astype(bf16),
                "ktT": pack(ktT[cols, :], 3).astype(bf16),
                "vh": pack(vh390, 3).astype(bf16),
                "rv": rv,
                "woT": pack(woT_full[cols, :], 3).astype(bf16),
                "cnt": C_s,
            })
    res = run_bass_kernel_spmd(nc, in_maps, list(range(NCORES)),
                               trace=os.environ.get("KERNEL_TRACE") == "1")
    _cache["tB"] = res.exec_time_ns
    out = np.zeros((B, K, D), np.float32)
    for b in range(B):
        outT = (res.results[2 * b]["outT"].astype(np.float32)
                + res.results[2 * b + 1]["outT"].astype(np.float32))
        out[b] = outT.T
    return out


# revision 31
# speedup vs baseline: 1.0993x; 1.0993x over previous
"""Trainium2 kernel for nn_LocalEncoder (BLT-style local encoder).

Key structural insight: every per-token quantity depends only on the token ID
(vocab=260), so the whole cross-attention collapses into vocab space:

  out_h(patch j) = sum_w C[w,j] * exp(S_h[w, qtok_j]) * vhat_h(w) / den
  den            = sum_w C[w,j] * exp(S_h[w, qtok_j])

with C[w,j] = count of tokens with id w inside patch j (host histogram),
S_h = khat_h^T qhat_h a (vocab x patch) score matrix, and qhat/khat/vhat the
vocab-space projection tables.  Device work per core is a handful of dense
vocab-sized matmuls -- no per-token gathers at all.

Pipeline:
  Kernel A (8 cores, DF split 8x384): zv partials = w2_slice @ silu(w1_slice @ embT)
  Host:     zv -> per-row boundary selection (stable by (-z, idx)) -> pos/pid,
            count matrix C[vocab, patch], qtok one-hot, folded weights
  Kernel B (8 cores = 4 seqs x 2 head-groups of 6): tables -> scores ->
            exp*count -> weighted-sum matmuls -> wo, partial outputs summed
            on host over the 2 head-groups.
"""

import os
import numpy as np
import ml_dtypes

import concourse.bass as bass
import concourse.bacc as bacc
import concourse.mybir as mybir
from concourse.tile import TileContext
from concourse.alu_op_type import AluOpType
from concourse.bass_utils import run_bass_kernel_spmd

F32 = mybir.dt.float32
F32R = mybir.dt.float32r
BF16 = mybir.dt.bfloat16
AFT = mybir.ActivationFunctionType
AX = mybir.AxisListType

B, L, D, V, K, H, HD = 4, 4096, 768, 260, 512, 12, 64
DF = 4 * D
VP = 384          # vocab padded to 3 partition chunks
RMS_EPS = 1e-5
NCORES = 8
FSL = DF // NCORES  # 384 f-rows per core in kernel A
DG = 384            # head-group width (6 heads x 64)

_cache = {}


# --------------------------------------------------------------------------- #
# Kernel A: per-core partial zv over a DF slice (fp32 matmuls for precision)
# --------------------------------------------------------------------------- #
def build_kernel_a():
    nc = bacc.Bacc("TRN2", target_bir_lowering=False, debug=False)
    embT_d = nc.dram_tensor("embT", [128, 6 * V], F32R, kind="ExternalInput")
    w1T_d = nc.dram_tensor("w1T", [128, 6 * FSL], F32R, kind="ExternalInput")
    bw_d = nc.dram_tensor("bw", [128, 6], F32, kind="ExternalInput")
    emb_d = nc.dram_tensor("emb", [128, 3 * D], BF16, kind="ExternalInput")
    wp_d = nc.dram_tensor("wp", [128, 6 * 192], BF16, kind="ExternalInput")
    wkp_d = nc.dram_tensor("wkp", [128, 6 * 96], BF16, kind="ExternalInput")
    zp_d = nc.dram_tensor("zp", [1, V], F32, kind="ExternalOutput")
    qv_d = nc.dram_tensor("qv", [128, 3 * 192], BF16, kind="ExternalOutput")
    kp_d = nc.dram_tensor("kp", [96, VP], BF16, kind="ExternalOutput")
    rv_d = nc.dram_tensor("rv", [128, 3], F32, kind="ExternalOutput")

    with TileContext(nc) as tc:
        with (
            tc.tile_pool(name="sb", bufs=1) as sb,
            tc.tile_pool(name="ps", bufs=2, space="PSUM") as ps,
        ):
            embT_t = sb.tile([128, 6 * V], F32R, tag="embT", name="embT_t")
            w1T_t = sb.tile([128, 6 * FSL], F32R, tag="w1T", name="w1T_t")
            bw = sb.tile([128, 6], F32, tag="bw", name="bw")
            emb_t = sb.tile([128, 3 * D], BF16, tag="emb", name="emb_t")
            wp_t = sb.tile([128, 6 * 192], BF16, tag="wp", name="wp_t")
            wkp_t = sb.tile([128, 6 * 96], BF16, tag="wkp", name="wkp_t")
            nc.sync.dma_start(bw[:, :], bw_d[:, :])
            for u in range(3):
                nc.sync.dma_start(emb_t[:, D * u:D * (u + 1)],
                                  emb_d[:, D * u:D * (u + 1)])
            for d in range(6):
                nc.sync.dma_start(embT_t[:, V * d:V * (d + 1)],
                                  embT_d[:, V * d:V * (d + 1)])
                nc.sync.dma_start(w1T_t[:, FSL * d:FSL * (d + 1)],
                                  w1T_d[:, FSL * d:FSL * (d + 1)])
            nc.sync.dma_start(wp_t[:, :], wp_d[:, :])
            nc.sync.dma_start(wkp_t[:, :], wkp_d[:, :])
            embT = [embT_t[:, V * d:V * (d + 1)] for d in range(6)]
            w1T = [w1T_t[:, FSL * d:FSL * (d + 1)] for d in range(6)]
            b1c = bw[:, 0:3]
            w2c = bw[:, 3:6]

            zp_ps = ps.tile([1, V], F32, tag="zp")
            for fi in range(3):
                y1p = ps.tile([128, V], F32, tag="y1")
                for d in range(6):
                    nc.tensor.matmul(
                        y1p[:, :], w1T_t[:, FSL * d + 128 * fi:FSL * d + 128 * (fi + 1)],
                        embT[d], start=(d == 0), stop=(d == 5),
                    )
                y1s = sb.tile([128, V], F32, tag="y1s")
                nc.scalar.activation(y1s[:, :], y1p[:, :], AFT.Silu,
                                     bias=b1c[:, fi:fi + 1])
                nc.tensor.matmul(zp_ps[:, :], w2c[:, fi:fi + 1], y1s[:, :],
                                 start=(fi == 0), stop=(fi == 2))
            zp_s = sb.tile([1, V], F32, tag="zps")
            nc.vector.tensor_copy(zp_s[:, :], zp_ps[:, :])
            nc.sync.dma_start(zp_d[:, :], zp_s[:, :])

            # bf16 copy of embT padded to VP columns per chunk
            embT_b = sb.tile([128, 6 * VP], BF16, tag="embTb", name="embT_b")
            nc.gpsimd.memset(embT_b[:, :], 0.0)
            for d in range(6):
                nc.vector.tensor_copy(embT_b[:, VP * d:VP * d + V], embT[d])

            # rv chain (bf16 emb squares -> rsqrt of mean)
            rv_t = sb.tile([128, 3], F32, tag="rv", name="rv_t")
            for u in range(3):
                sq = sb.tile([128, D], BF16, tag="sq", name="sq", bufs=2)
                nc.gpsimd.tensor_tensor(sq[:, :], emb_t[:, D * u:D * (u + 1)],
                                        emb_t[:, D * u:D * (u + 1)], AluOpType.mult)
                msq = sb.tile([128, 1], F32, tag="msq", name="msq", bufs=2)
                nc.vector.tensor_reduce(msq[:, :], sq[:, :], AX.X, AluOpType.add)
                tn = sb.tile([128, 1], F32, tag="tn", name="tn", bufs=2)
                nc.vector.tensor_scalar(tn[:, :], msq[:, :], 1.0 / D, RMS_EPS,
                                        AluOpType.mult, AluOpType.add)
                tr = sb.tile([128, 1], F32, tag="tr", name="tr", bufs=2)
                nc.vector.reciprocal(tr[:, :], tn[:, :])
                nc.scalar.activation(rv_t[:, u:u + 1], tr[:, :], AFT.Sqrt)
            nc.sync.dma_start(rv_d[:, :], rv_t[:, :])

            # table piece: qv = (emb @ w_cols) * rv ; kp = w_rowsT @ embT
            qv_s = sb.tile([128, 3 * 192], BF16, tag="qvs", name="qv_s")
            for u in range(3):
                qvp = ps.tile([128, 192], F32, tag="t192", name="qvp")
                for d in range(6):
                    nc.tensor.matmul(qvp[:, :],
                                     embT_b[:, VP * d + 128 * u:VP * d + 128 * (u + 1)],
                                     wp_t[:, 192 * d:192 * (d + 1)],
                                     start=(d == 0), stop=(d == 5))
                nc.vector.tensor_scalar(qv_s[:, 192 * u:192 * (u + 1)], qvp[:, :],
                                        rv_t[:, u:u + 1], None, AluOpType.mult)
            nc.sync.dma_start(qv_d[:, :], qv_s[:, :])
            kpp = ps.tile([96, VP], F32, tag="kpp", name="kpp")
            for d in range(6):
                nc.tensor.matmul(kpp[:, :], wkp_t[:, 96 * d:96 * (d + 1)],
                                 embT_b[:, VP * d:VP * (d + 1)],
                                 start=(d == 0), stop=(d == 5))
            kp_s = sb.tile([96, VP], BF16, tag="kps", name="kp_s")
            nc.vector.tensor_copy(kp_s[:, :], kpp[:, :])
            nc.sync.dma_start(kp_d[:, :], kp_s[:, :])

    nc.compile()
    return nc


def run_kernel_a(inputs, emb_s, wqT_full, wkT_full, wvT_full, pack):
    if "A" not in _cache:
        _cache["A"] = build_kernel_a()
    nc = _cache["A"]
    bf16 = ml_dtypes.bfloat16
    embT = inputs["embed_W"].astype(np.float32).T          # [768, 260]
    embT_r = np.ascontiguousarray(
        embT.reshape(6, 128, V).transpose(1, 0, 2).reshape(128, 6 * V))
    w1 = inputs["bp_w1"].astype(np.float32)
    b1 = inputs["bp_b1"].astype(np.float32)
    w2 = inputs["bp_w2"].astype(np.float32)[0]
    in_maps = []
    for c in range(NCORES):
        sl = slice(c * FSL, (c + 1) * FSL)
        w1T_r = np.ascontiguousarray(
            w1[sl].T.reshape(6, 128, FSL).transpose(1, 0, 2).reshape(128, 6 * FSL))
        bw = np.zeros((128, 6), np.float32)
        bw[:, 0:3] = b1[sl].reshape(3, 128).T
        bw[:, 3:6] = w2[sl].reshape(3, 128).T
        if c < 4:
            wp = wqT_full[:, 192 * c:192 * (c + 1)]
        else:
            wp = wvT_full[:, 192 * (c - 4):192 * (c - 3)]
        in_maps.append({
            "embT": embT_r, "w1T": w1T_r, "bw": bw, "emb": emb_s,
            "wp": pack(wp, 6).astype(bf16),
            "wkp": pack(wkT_full[:, 96 * c:96 * (c + 1)], 6).astype(bf16),
        })
    res = run_bass_kernel_spmd(nc, in_maps, list(range(NCORES)),
                               trace=os.environ.get("KERNEL_TRACE") == "1")
    _cache["tA"] = res.exec_time_ns
    zv = np.zeros(V, np.float64)
    for c in range(NCORES):
        zv += res.results[c]["zp"][0].astype(np.float64)
    zv += inputs["bp_b2"].astype(np.float64)[0]

    def unpack(a, nchunk):
        p, nc_ = a.shape
        c = nc_ // nchunk
        return a.reshape(p, nchunk, c).transpose(1, 0, 2).reshape(nchunk * p, c)

    qhat = np.zeros((VP, D), np.float32)
    vhat = np.zeros((VP, D), np.float32)
    ktT = np.zeros((D, VP), np.float32)
    for c in range(NCORES):
        r = res.results[c]
        qv = unpack(r["qv"].astype(np.float32), 3)
        if c < 4:
            qhat[:, 192 * c:192 * (c + 1)] = qv
        else:
            vhat[:, 192 * (c - 4):192 * (c - 3)] = qv
        ktT[96 * c:96 * (c + 1), :] = r["kp"].astype(np.float32)
    rv = res.results[0]["rv"]
    return zv.astype(np.float32), qhat, vhat, ktT, rv


# --------------------------------------------------------------------------- #
# Host boundary logic
# --------------------------------------------------------------------------- #
def boundary_plan(zv, tokens):
    """Reproduce reference top-k (stable ties by index) + patch structure."""
    zt = zv[tokens]  # [B, L]
    pos = np.zeros((B, K), np.int64)
    for b in range(B):
        key = zt[b].astype(np.float64).copy()
        key[0] = np.inf  # position 0 forced boundary (logprob set to 0 = max)
        order = np.lexsort((np.arange(L), -key))
        pos[b] = np.sort(order[:K])
    pid = (pos[:, None, :] <= np.arange(L)[None, :, None]).sum(-1) - 1  # [B, L]
    return pos, pid


# --------------------------------------------------------------------------- #
# Kernel B: count-matrix vocab-space cross attention, 6 heads per core
# --------------------------------------------------------------------------- #
def strided3(ap, n, w, stride, offset=0):
    """[128, *] AP viewed as [128, n, w] blocks at `offset` with block stride."""
    ps = ap.ap[0]
    return bass.AP(ap.tensor, ap.offset + offset,
                   [list(ps), [stride, n], [1, w]])


def build_kernel_b():
    nc = bacc.Bacc("TRN2", target_bir_lowering=False, debug=False)
    qgt_d = nc.dram_tensor("qgt", [128, 3 * K], BF16, kind="ExternalInput")
    ktT_d = nc.dram_tensor("ktT", [128, 3 * VP], BF16, kind="ExternalInput")
    vh_d = nc.dram_tensor("vh", [128, 3 * 390], BF16, kind="ExternalInput")
    rv_d = nc.dram_tensor("rv", [128, 3], F32, kind="ExternalInput")
    woT_d = nc.dram_tensor("woT", [128, 3 * D], BF16, kind="ExternalInput")
    c_d = nc.dram_tensor("cnt", [128, 3 * K], BF16, kind="ExternalInput")
    outT_d = nc.dram_tensor("outT", [D, K], BF16, kind="ExternalOutput")

    with TileContext(nc) as tc:
        with (
            tc.tile_pool(name="sb", bufs=1) as sb,
            tc.tile_pool(name="ps", bufs=2, space="PSUM") as ps,
        ):
            # ---- loads ----
            qgt_t = sb.tile([128, 3 * K], BF16, tag="qgt", name="qgt_t")
            ktT_t = sb.tile([128, 3 * VP], BF16, tag="ktT", name="ktT_t")
            vh_t = sb.tile([128, 3 * 390], BF16, tag="vh", name="vh_t")
            rv_t = sb.tile([128, 3], F32, tag="rvt", name="rv_t")
            c_t = sb.tile([128, 3 * K], BF16, tag="ct", name="c_t")
            wo_t = sb.tile([128, 3 * D], BF16, tag="wot", name="wo_t")
            nc.sync.dma_start(rv_t[:, :], rv_d[:, :])
            for u in range(3):
                nc.sync.dma_start(qgt_t[:, K * u:K * (u + 1)],
                                  qgt_d[:, K * u:K * (u + 1)])
                nc.sync.dma_start(ktT_t[:, VP * u:VP * (u + 1)],
                                  ktT_d[:, VP * u:VP * (u + 1)])
                nc.sync.dma_start(vh_t[:, 390 * u:390 * (u + 1)],
                                  vh_d[:, 390 * u:390 * (u + 1)])
                nc.sync.dma_start(c_t[:, K * u:K * (u + 1)],
                                  c_d[:, K * u:K * (u + 1)])
            for u in range(3):
                nc.sync.dma_start(wo_t[:, D * u:D * (u + 1)],
                                  woT_d[:, D * u:D * (u + 1)])
            ktT3 = [ktT_t[:, VP * r:VP * (r + 1)] for r in range(3)]
            vhat3 = [vh_t[:, 390 * u:390 * (u + 1)] for u in range(3)]
            C3 = [c_t[:, K * u:K * (u + 1)] for u in range(3)]
            woT3 = [wo_t[:, D * u:D * (u + 1)] for u in range(3)]
            rv3 = [rv_t[:, u:u + 1] for u in range(3)]
            ones64 = sb.tile([128, 64], BF16, tag="ones64")
            nc.gpsimd.memset(ones64[:, :], 1.0)

            # qgT DMA'd directly (host gathers qhat rows by qtok)
            qgT3 = [qgt_t[:, K * r:K * (r + 1)] for r in range(3)]

            # ---- attention: per head scores -> exp*count -> num/den ----
            prT3 = [sb.tile([128, K], BF16, tag=f"prT{r}", name=f"prT{r}") for r in range(3)]
            for h in range(6):
                r, off = h // 2, 64 * (h % 2)
                nm = ps.tile([128, K], F32, tag="num", name="nm", bufs=2)
                for w in range(3):
                    sp = ps.tile([128, K], F32, tag="s3", name="sp", bufs=3)
                    nc.tensor.matmul(sp[:, :],
                                     ktT3[r][off:off + 64, 128 * w:128 * (w + 1)],
                                     qgT3[r][off:off + 64, :],
                                     start=True, stop=True)
                    ex = sb.tile([128, K], BF16, tag="ex", name="ex", bufs=4)
                    nc.scalar.activation(ex[:, :], sp[:, :], AFT.Exp,
                                         scale=rv3[w])
                    xt = sb.tile([128, K], BF16, tag="xt", name="xt", bufs=4)
                    nc.vector.tensor_tensor(xt[:, :], ex[:, :], C3[w][:, :],
                                            AluOpType.mult)
                    nc.tensor.matmul(nm[0:65, :], vhat3[w][:, 65 * h:65 * h + 65],
                                     xt[:, :], start=(w == 0), stop=(w == 2))
                rdf = sb.tile([128, K], F32, tag="rdf", name="rdf", bufs=3)
                nc.vector.reciprocal_approx_fast(rdf[:, :], nm[:, :])
                rd = sb.tile([128, K], BF16, tag="rd", name="rd", bufs=3)
                nc.vector.tensor_copy(rd[64:65, :], rdf[64:65, :])
                pp = ps.tile([128, K], F32, tag="pp", name="pp", bufs=1)
                nc.tensor.matmul(pp[0:64, :], ones64[64:65, :], rd[64:65, :],
                                 start=True, stop=True)
                nmsb = sb.tile([64, K], BF16, tag="nmsb", name="nmsb", bufs=3)
                nc.vector.tensor_copy(nmsb[:, :], nm[0:64, :])
                if off == 0:
                    nc.vector.tensor_tensor(prT3[r][0:64, :], nmsb[:, :],
                                            pp[0:64, :], AluOpType.mult)
                else:
                    po = sb.tile([64, K], BF16, tag="po", name="po", bufs=2)
                    nc.vector.tensor_tensor(po[:, :], nmsb[:, :], pp[0:64, :],
                                            AluOpType.mult)
                    nc.sync.dma_start(prT3[r][64:128, :], po[:, :])

            # ---- wo projection (transposed output) ----
            for m in range(6):
                op = ps.tile([128, K], F32, tag="w512", name="op")
                for kc in range(3):
                    nc.tensor.matmul(op[:, :], woT3[kc][:, 128 * m:128 * (m + 1)],
                                     prT3[kc][:, :], start=(kc == 0), stop=(kc == 2))
                ot = sb.tile([128, K], BF16, tag="ot", name="ot", bufs=2)
                nc.vector.tensor_copy(ot[:, :], op[:, :])
                nc.sync.dma_start(outT_d[128 * m:128 * (m + 1), :], ot[:, :])
    nc.compile()
    return nc


# --------------------------------------------------------------------------- #
# top-level
# --------------------------------------------------------------------------- #
def kernel(tokens, embed_W, bp_w1, bp_b1, bp_w2, bp_b2, wq, wk, wv, wo,
           qnorm_w, kvnorm_w, k_patches):
    tokens = np.asarray(tokens).astype(np.int64)
    inputs = dict(tokens=tokens, embed_W=embed_W, bp_w1=bp_w1, bp_b1=bp_b1,
                  bp_w2=bp_w2, bp_b2=bp_b2)
    bf16 = ml_dtypes.bfloat16

    def pack(a, nchunk):
        """[nchunk*128, C] -> [128, nchunk*C] chunk-column layout."""
        n, c = a.shape
        assert n == nchunk * 128
        return np.ascontiguousarray(
            a.reshape(nchunk, 128, c).transpose(1, 0, 2).reshape(128, nchunk * c))

    embp = np.zeros((VP, D), np.float32)
    embp[:V] = embed_W.astype(np.float32)
    emb_s = pack(embp, 3).astype(bf16)
    wqT_full = np.ascontiguousarray(
        (wq.astype(np.float32) * qnorm_w.astype(np.float32)[None, :]).T / 8.0)
    wkT_full = np.ascontiguousarray(
        (wk.astype(np.float32) * kvnorm_w.astype(np.float32)[None, :]).T)
    wvT_full = np.ascontiguousarray(
        (wv.astype(np.float32) * kvnorm_w.astype(np.float32)[None, :]).T)
    woT_full = np.ascontiguousarray(wo.astype(np.float32).T)

    zv, qhat, vhat, ktT, rv = run_kernel_a(
        inputs, emb_s, wqT_full, wkT_full, wvT_full, pack)
    pos, pid = boundary_plan(zv, tokens)
    qtokp = np.take_along_axis(tokens, pos, 1)  # [B, K] boundary token ids

    if "B" not in _cache:
        _cache["B"] = build_kernel_b()
    nc = _cache["B"]

    qhat_b = qhat.astype(bf16).astype(np.float32)
    in_maps = []
    for b in range(B):
        C = np.zeros((VP, K), np.float32)
        np.add.at(C, (tokens[b], pid[b]), 1.0)
        C_s = pack(C, 3).astype(bf16)
        qg_b = qhat_b[qtokp[b]]                    # [K, 768] gather
        for g in range(2):
            cols = slice(DG * g, DG * (g + 1))
            vh390 = np.zeros((VP, 390), np.float32)
            for h in range(6):
                vh390[:, 65 * h:65 * h + 64] = vhat[:, DG * g + 64 * h:DG * g + 64 * (h + 1)]
                vh390[:, 65 * h + 64] = 1.0
            in_maps.append({
                "qgt": pack(np.ascontiguousarray(qg_b[:, cols].T), 3).astype(bf16),
                "ktT": pack(ktT[cols, :], 3).astype(bf16),
                "vh": pack(vh390, 3).astype(bf16),
                "rv": rv,
                "woT": pack(woT_full[cols, :], 3).astype(bf16),
                "cnt": C_s,
            })
    res = run_bass_kernel_spmd(nc, in_maps, list(range(NCORES)),
                               trace=os.environ.get("KERNEL_TRACE") == "1")
    _cache["tB"] = res.exec_time_ns
    out = np.zeros((B, K, D), np.float32)
    for b in range(B):
        outT = (res.results[2 * b]["outT"].astype(np.float32)
                + res.results[2 * b + 1]["outT"].astype(np.float32))
        out[b] = outT.T
    return out
